# revision 1
# baseline (speedup 1.0000x reference)
"""Trainium2 Bass kernel for nn_Attention_21792664060632.

GQA attention (32 q heads, 8 kv heads, d=64, s=2048, hidden=2048, causal,
interleaved RoPE with random cos/sin) sharded tensor-parallel over 8
NeuronCores: core c owns q heads {c, c+8, c+16, c+24} (all of which use kv
head c under the reference's jnp.tile GQA mapping) plus kv head c.  Each
core computes a partial output x-projection -> rope -> attention -> @Wo_rows
and the host sums the 8 partials.

Per-core dataflow (all matmuls in float32r, transposes in exact float32):
  A. x row-chunk [128, 2048] -> PE-transpose -> xT tiles (hidden on
     partitions)
  B. QKV projection: psum[128, 384] = sum_k xT[k].T @ Wcat[k]
     (Wcat = [Wq 4 heads | Wk | Wv] columns), RoPE applied in natural
     layout (pairs on free dim), V kept natural with a ones column
  C. PE-transpose rope'd Q,K -> QT pairs [128, 2048] (2 heads stacked),
     KT [128, 2048] (kv head replicated in both partition halves)
  D. per head-pair, per 512-wide q tile: S^T[k,q] matmuls (two heads run
     concurrently in disjoint PE row groups), exp on ScalarE, causal mask,
     AV matmuls with ones-augmented V (M=65) accumulate att output and
     softmax denominators together; normalize via reciprocal + gpsimd
     partition-broadcast
  E. out_partial[s,:] = sum_pair avT_pair.T @ Wo_pair rows
"""

import sys

sys.path.insert(0, "/opt/trn_rl_repo")

import numpy as np

HEADS, KV_HEADS, HEAD_DIM = 32, 8, 64
S, HID = 2048, 2048
NCORES = 8
SC = S // 128  # 16 s-chunks
KC = HID // 128  # 16 hidden-chunks
NQT = S // 512  # 4 q-tiles

_CACHE = {}


def _build_nc():
    import concourse.bacc as bacc
    import concourse.mybir as mybir
    import concourse.tile as tile
    from concourse.masks import make_identity

    F32 = mybir.dt.float32
    F32R = mybir.dt.float32r
    EXP = mybir.ActivationFunctionType.Exp
    MULT = mybir.AluOpType.mult
    SUB = mybir.AluOpType.subtract
    ADD = mybir.AluOpType.add

    nc = bacc.Bacc("TRN2", target_bir_lowering=False, debug=False)

    XT = nc.dram_tensor("xt", [HID, S], F32R, kind="ExternalInput")
    WCAT = nc.dram_tensor("wcat", [HID, 384], F32R, kind="ExternalInput")
    WO = nc.dram_tensor("wo", [256, HID], F32R, kind="ExternalInput")
    COS = nc.dram_tensor("cos", [S, 32], F32, kind="ExternalInput")
    SIN = nc.dram_tensor("sin", [S, 32], F32, kind="ExternalInput")
    OUT = nc.dram_tensor("out", [S, HID], F32, kind="ExternalOutput")

    with tile.TileContext(nc) as tc:
        with (
            tc.tile_pool(name="const", bufs=1) as const,
            tc.tile_pool(name="weights", bufs=1) as wpool,
            tc.tile_pool(name="persist", bufs=1) as persist,
        ):
            ident = const.tile([128, 128], F32)
            make_identity(nc, ident[:])
            mask01 = const.tile([128, 128], F32)
            nc.gpsimd.memset(mask01[:], 1.0)
            # keep only q >= k: free index (q) >= partition index (k)
            nc.gpsimd.affine_select(
                out=mask01[:], in_=mask01[:],
                compare_op=mybir.AluOpType.is_ge,
                fill=0.0, base=0,
                pattern=[[1, 128]], channel_multiplier=-1,
            )
            mask01r = const.tile([128, 128], F32R)
            nc.vector.tensor_copy(mask01r[:], mask01[:])
            ones_col = const.tile([128, 1], F32)
            nc.vector.memset(ones_col[:], 1.0)
            ones_r = const.tile([128, 1], F32R)
            nc.vector.tensor_copy(ones_r[:], ones_col[:])
            cos_sb = const.tile([128, SC, 32], F32)
            sin_sb = const.tile([128, SC, 32], F32)
            nc.gpsimd.dma_start(cos_sb[:], COS[:].rearrange("(c p) f -> p c f", p=128))
            nc.gpsimd.dma_start(sin_sb[:], SIN[:].rearrange("(c p) f -> p c f", p=128))

            wcat_sb = wpool.tile([128, KC, 384], F32R)
            wo_sb = wpool.tile([128, 2, HID], F32R)
            for c in range(KC):
                nc.gpsimd.dma_start(
                    wcat_sb[:, c, :], WCAT[c * 128:(c + 1) * 128, :]
                )
            for c in range(2):
                nc.gpsimd.dma_start(wo_sb[:, c, :], WO[c * 128:(c + 1) * 128, :])

            # persistent transposed activations (f32r for fast matmul)
            qt0 = persist.tile([128, S], F32R)  # heads pair 0 (rows 0:64, 64:128)
            qt1 = persist.tile([128, S], F32R)  # heads pair 1
            kt = persist.tile([128, S], F32R)  # kv head replicated in both halves
            v_sb = persist.tile([128, SC, 65], F32R)  # V natural + ones column
            avt0 = persist.tile([128, S], F32R)  # normalized attn out, pair 0
            avt1 = persist.tile([128, S], F32R)
            qts = [qt0, qt1]
            avts = [avt0, avt1]

            # ---- interleaved pipeline: per group g of 4 s-chunks:
            #   ABC(si in group) -> attention(qj=g, both pairs) -> out(si group)
            with (
                tc.tile_pool(name="abc", bufs=3) as abc,
                tc.tile_pool(name="ropet", bufs=3) as ropet,
                tc.tile_pool(name="pd", bufs=3) as pd,
                tc.tile_pool(name="nrm", bufs=1) as nrm,
                tc.tile_pool(name="pe", bufs=3) as pe,
                tc.tile_pool(name="big", bufs=3, space="PSUM") as big,
                tc.tile_pool(name="psav", bufs=1, space="PSUM") as psav,
            ):
                xt_grps = {}

                def prefetch_x(grp_idx):
                    xg = abc.tile(
                        [128, KC, 512], F32R, tag="xtg", bufs=2,
                        name=f"xtg_{grp_idx}",
                    )
                    src_cols = XT[:, grp_idx * 512:(grp_idx + 1) * 512]
                    view = src_cols.rearrange("(c p) s -> p c s", p=128)
                    for q in range(4):
                        nc.sync.dma_start(
                            xg[:, q * 4:(q + 1) * 4, :], view[:, q * 4:(q + 1) * 4, :]
                        )
                    xt_grps[grp_idx] = xg

                # deferred rot-transpose work: si -> rot tile
                pending_rot = []

                def emit_rot_transpose():
                    if not pending_rot:
                        return
                    si, rot = pending_rot.pop(0)
                    tq = big.tile([128, 1024], F32, tag="big", name=f"tq_{si}")
                    nc.tensor.transpose(tq[:, 0:128], rot[:, 0:128], ident[:])
                    nc.tensor.transpose(tq[:, 128:256], rot[:, 128:256], ident[:])
                    nc.tensor.transpose(tq[0:64, 256:384], rot[:, 256:320], ident[:])
                    ss = slice(si * 128, (si + 1) * 128)
                    nc.vector.tensor_copy(qts[0][:, ss], tq[:, 0:128])
                    nc.vector.tensor_copy(qts[1][:, ss], tq[:, 128:256])
                    nc.vector.tensor_copy(kt[0:64, ss], tq[0:64, 256:384])
                    nc.vector.tensor_copy(kt[64:128, ss], tq[0:64, 256:384])

                def abc_stage(si):
                    xg = xt_grps[si // 4]
                    so = (si % 4) * 128
                    qkv_t = big.tile([128, 1024], F32, tag="big", name=f"qkv_{si}")
                    qkv = qkv_t[:, 0:384]
                    for kc in range(KC):
                        nc.tensor.matmul(
                            qkv, xg[:, kc, so:so + 128], wcat_sb[:, kc, :],
                            start=(kc == 0), stop=(kc == KC - 1),
                        )
                    # deferred rot-transpose of the PREVIOUS si runs while this
                    # si's rope chain completes on DVE/GpSimd
                    emit_rot_transpose()
                    # RoPE in natural layout (pairs on free dim)
                    qk = qkv_t[:, 0:320].rearrange(
                        "p (gr i t) -> p gr i t", gr=5, t=2
                    )
                    q1 = qk[:, :, :, 0]
                    q2 = qk[:, :, :, 1]
                    cs = cos_sb[:, si, None, :].to_broadcast([128, 5, 32])
                    sn = sin_sb[:, si, None, :].to_broadcast([128, 5, 32])
                    t1 = ropet.tile([128, 5, 32], F32, tag="t1")
                    t2 = ropet.tile([128, 5, 32], F32, tag="t2")
                    t3 = ropet.tile([128, 5, 32], F32, tag="t3")
                    t4 = ropet.tile([128, 5, 32], F32, tag="t4")
                    nc.vector.tensor_tensor(t1[:], q1, cs, MULT)
                    nc.vector.tensor_tensor(t2[:], q2, sn, MULT)
                    nc.vector.tensor_tensor(t3[:], q1, sn, MULT)
                    nc.vector.tensor_tensor(t4[:], q2, cs, MULT)
                    rot = ropet.tile([128, 320], F32, tag="rot", bufs=3)
                    rv = rot[:].rearrange("p (gr i t) -> p gr i t", gr=5, t=2)
                    nc.vector.tensor_tensor(rv[:, :, :, 0], t1[:], t2[:], SUB)
                    nc.vector.tensor_tensor(rv[:, :, :, 1], t3[:], t4[:], ADD)
                    # V natural + ones column
                    nc.vector.tensor_copy(v_sb[:, si, 0:64], qkv_t[:, 320:384])
                    nc.vector.tensor_copy(v_sb[:, si, 64:65], ones_r[:])
                    pending_rot.append((si, rot))

                def attention_stage(qj, fillers=()):
                    fillers = list(fillers)
                    n_units = 2 * (4 * qj + 4)
                    stride = max(1, n_units // (len(fillers) + 1)) if fillers else 0
                    unit_idx = 0
                    q0 = qj * 512
                    kimax = 4 * qj + 3
                    for pr in range(2):
                        qt = qts[pr]
                        av_a = psav.tile([65, 512], F32, tag="ava")
                        av_b = psav.tile([65, 512], F32, tag="avb")
                        avs = (av_a, av_b)
                        for ki in range(kimax + 1):
                            d = ki - 4 * qj
                            qoff = 0 if d < 0 else d * 128
                            st = big.tile([128, 1024], F32, tag="big")
                            for h in range(2):
                                hp = h * 64
                                nc.tensor.matmul(
                                    st[:, h * 512 + qoff:h * 512 + 512],
                                    kt[hp:hp + 64, ki * 128:(ki + 1) * 128],
                                    qt[hp:hp + 64, q0 + qoff:q0 + 512],
                                    start=True, stop=True,
                                )
                            p = pd.tile([128, 1024], F32R, tag="p", bufs=4)
                            if d < 0:
                                nc.scalar.activation(p[:], st[:], EXP, scale=0.125)
                            for h in range(2):
                                o = h * 512 + qoff
                                if d >= 0:
                                    nc.scalar.activation(
                                        p[:, o:h * 512 + 512],
                                        st[:, o:h * 512 + 512],
                                        EXP, scale=0.125,
                                    )
                                    nc.vector.tensor_tensor(
                                        p[:, o:o + 128], p[:, o:o + 128],
                                        mask01r[:], MULT,
                                    )
                                nc.tensor.matmul(
                                    avs[h][:, qoff:512],
                                    v_sb[:, ki, :],
                                    p[:, o:(h + 1) * 512],
                                    start=(ki == 0), stop=(ki == kimax),
                                )
                            unit_idx += 1
                            if fillers and stride and unit_idx % stride == 0:
                                fillers.pop(0)()
                        # normalize: row 64 of av psum is the denominator
                        for h in range(2):
                            hp = h * 64
                            den = nrm.tile([1, 512], F32, tag=f"den{h}")
                            nc.scalar.copy(den[:], avs[h][64:65, :])
                            rec = nrm.tile([1, 512], F32, tag=f"rec{h}")
                            nc.vector.reciprocal_approx_fast(rec[:], den[:])
                            bc = nrm.tile([64, 512], F32, tag=f"bc{h}")
                            nc.gpsimd.partition_broadcast(bc[:], rec[0:1, :])
                            nc.vector.tensor_tensor(
                                avts[pr][hp:hp + 64, qj * 512:(qj + 1) * 512],
                                avs[h][0:64, :], bc[:], MULT,
                            )

                def out_stage(g):
                    for si in range(4 * g, 4 * g + 4):
                        for njp in range(2):
                            ops_t = big.tile([128, 1024], F32, tag="big")
                            for nj2 in range(2):
                                nj = njp * 2 + nj2
                                for prx in range(2):
                                    nc.tensor.matmul(
                                        ops_t[:, nj2 * 512:(nj2 + 1) * 512],
                                        avts[prx][:, si * 128:(si + 1) * 128],
                                        wo_sb[:, prx, nj * 512:(nj + 1) * 512],
                                        start=(prx == 0), stop=(prx == 1),
                                    )
                            osb = pe.tile([128, 1024], F32, tag="ob")
                            if njp == 0:
                                nc.vector.tensor_copy(osb[:], ops_t[:])
                            else:
                                nc.scalar.copy(osb[:], ops_t[:])
                            nc.gpsimd.dma_start(
                                OUT[si * 128:(si + 1) * 128,
                                    njp * 1024:(njp + 1) * 1024],
                                osb[:],
                            )

                # schedule: ABC(0) | for g: D(g) interleaved with ABC(g+1),
                # then E(g)
                prefetch_x(0)
                for si in range(4):
                    abc_stage(si)
                emit_rot_transpose()
                for g in range(4):
                    if g < 3:
                        prefetch_x(g + 1)
                        fillers = [
                            (lambda si=si: abc_stage(si))
                            for si in range(4 * g + 4, 4 * g + 8)
                        ]
                    else:
                        fillers = []
                    attention_stage(g, fillers[:3])
                    for f in fillers[3:]:
                        f()
                    out_stage(g)
                    emit_rot_transpose()

    nc.compile()
    return nc


def _shard_inputs(x, cos, sin, Wq, Wk, Wv, Wo):
    """Build the 8 per-core input maps (tensor-parallel by head groups)."""
    xt = np.ascontiguousarray(x.T)
    in_maps = []
    for c in range(NCORES):
        heads = [c, c + 8, c + 16, c + 24]
        wq_cols = np.concatenate(
            [Wq[:, h * 64:(h + 1) * 64] for h in heads], axis=1
        )
        wcat = np.concatenate(
            [wq_cols, Wk[:, c * 64:(c + 1) * 64], Wv[:, c * 64:(c + 1) * 64]],
            axis=1,
        ).astype(np.float32)
        wo_rows = np.concatenate(
            [Wo[h * 64:(h + 1) * 64, :] for h in heads], axis=0
        ).astype(np.float32)
        in_maps.append(
            {
                "xt": xt,
                "wcat": np.ascontiguousarray(wcat),
                "wo": np.ascontiguousarray(wo_rows),
                "cos": np.ascontiguousarray(cos),
                "sin": np.ascontiguousarray(sin),
            }
        )
    return in_maps


def run(inputs, trace=False):
    """Run on all 8 cores; returns (full_output [1,S,HID], BassKernelResults)."""
    from concourse.bass_utils import run_bass_kernel_spmd

    x = np.asarray(inputs["x"], dtype=np.float32)[0]
    cos = np.asarray(inputs["cos"], dtype=np.float32)
    sin = np.asarray(inputs["sin"], dtype=np.float32)
    Wq = np.asarray(inputs["Wq"], dtype=np.float32)
    Wk = np.asarray(inputs["Wk"], dtype=np.float32)
    Wv = np.asarray(inputs["Wv"], dtype=np.float32)
    Wo = np.asarray(inputs["Wo"], dtype=np.float32)

    if "nc" not in _CACHE:
        _CACHE["nc"] = _build_nc()
    nc = _CACHE["nc"]

    in_maps = _shard_inputs(x, cos, sin, Wq, Wk, Wv, Wo)
    res = run_bass_kernel_spmd(
        nc, in_maps, core_ids=list(range(NCORES)), trace=trace
    )
    out = np.zeros((S, HID), dtype=np.float32)
    for r in res.results:
        out += r["out"]
    return out.reshape(1, S, HID), res


def kernel(**inputs) -> np.ndarray:
    out, _ = run(inputs, trace=False)
    return out

